# revision 14
# baseline (speedup 1.0000x reference)
"""Contrastive loss (SimCLR-style, masked-diagonal logsumexp) on 8 Trainium2
NeuronCores via Bass/Tile.

Math (matches the jax reference):
    a = anchor / ||anchor||_row ; p = positive / ||positive||_row
    F = concat([a, p])                         # [R=2B, D]
    sim = (F F^T) / T with diagonal masked
    lse_i = log(sum_{j!=i} exp(sim_ij))
    pos_i = <a_i, p_i> / T  (duplicated for both halves)
    loss = sum_i (lse_i - pos_i) * lab_i / max(sum_i lab_i, 1)

exp(sim) is symmetric, so only the upper triangle of the 16x16 grid of
512x512 blocks is computed (136 of 256 blocks): each block (I, J) yields
row-sum partials for chunk I (ACT Exp accum) and, when I != J, column-sum
partials for chunk J (ones-vector PE matmul over the exp tile).

Distribution (uniform SPMD stream, zero collectives): core c owns chunk
rows c and c+8. Step (h, g) computes block (I, (I+g) mod 16) with
I = c + 8h, g = 0..8 for h=0 and g = 0..7 for h=1 — a circular-gap
schedule that covers every unordered block pair exactly once and gives
every core the identical 17-step instruction shape. All per-core
variation lives in the host-side chunk roll (slot s holds global chunk
(c+s) mod 16), so the stationary operands sit at fixed slots 0 and 8.

Features are L2-normalized on the host, scaled by 16 and quantized to
fp8e4 (e4m3); matmuls run in DoubleRow perf mode (2 k-subtiles per
instruction, 2x PE throughput). sim = G * (1/T)/256 rides the ACT Exp
scale. The diagonal of the two diag blocks is pushed to ~-14 in sim
units by a DVE mask subtract (exp -> ~6e-7, negligible like the
reference's exp(-1e9) = 0). Exp tiles are written back as fp8 m-tile
pairs so each column-sum is 2 DoubleRow matmuls against a ones vector
(row sums come from the ACT accumulator, which sums in f32 pre-cast).

The device ships raw per-step row/column-sum partials (~66 KB/core)
eagerly per group; the host un-rolls them, adds across cores, and
finishes the scalar: loss = sum(lab*(ln(rowsum) - pos))/num_pos.
"""

import os
import numpy as np
import ml_dtypes


# ---------------------------------------------------------------- config ----
class CFG:
    B = 4096
    D = 1024
    NC = 8           # cores
    JW = 512         # chunk width (one PSUM bank of f32)
    JC = 16          # number of row/col chunks (R / JW)
    KC = 8           # k-subtiles of 128
    TEMP = 0.07
    SCALE = 16.0     # fp8 pre-scale; G = SCALE^2 * cos
    MASKSUB = 512.0  # subtracted from G on the diagonal (~2x diag value)
    MODE = "f8dr"    # 'f8dr' (fp8 DoubleRow) | 'bf16'

    @property
    def R(self):
        return 2 * self.B

    # step table: t -> (h, g, slot, isdiag); slots are per-core rolled.
    @property
    def steps(self):
        out = []
        for t in range(9):
            out.append((0, t, t, t == 0))
        for t in range(8):
            out.append((1, t, 8 + t, t == 0))
        return out

    # emission groups: lists of step indices sharing one stationary slot
    @property
    def groups(self):
        return [
            (0, [0, 1, 2]), (0, [3, 4, 5]), (0, [6, 7, 8]),
            (8, [9, 10, 11]), (8, [12, 13, 14]), (8, [15, 16]),
        ]


_BUILD_CACHE = {}


# ----------------------------------------------------------------- build ----
def build_nc(cfg: CFG):
    import concourse.bass as bass  # noqa: F401  (AP helpers live here)
    import concourse.tile as tile
    from contextlib import ExitStack
    from concourse import bacc, mybir

    f32 = mybir.dt.float32
    bf16 = mybir.dt.bfloat16
    f8 = mybir.dt.float8e4
    Act = mybir.ActivationFunctionType
    Alu = mybir.AluOpType

    JW, JC, KC = cfg.JW, cfg.JC, cfg.KC
    fp8 = cfg.MODE == "f8dr"
    mmdt = f8 if fp8 else bf16
    K2 = KC // 2 if fp8 else KC           # matmul k-steps per block
    perf = mybir.MatmulPerfMode.DoubleRow if fp8 else None
    sc = (1.0 / cfg.TEMP) / (cfg.SCALE * cfg.SCALE)
    steps = cfg.steps
    nsteps = len(steps)

    nc = bacc.Bacc("TRN2", target_bir_lowering=False, debug=False,
                   num_devices=cfg.NC)

    fch = nc.dram_tensor("fchunks", [JC, 128, KC, JW], mmdt,
                         kind="ExternalInput").ap()
    maskd = nc.dram_tensor("maskd", [128, 4, JW], bf16,
                           kind="ExternalInput").ap()
    rs_out = nc.dram_tensor("rs_out", [128, nsteps, 4], f32,
                            kind="ExternalOutput").ap()
    cs_out = nc.dram_tensor("cs_out", [1, nsteps * JW], f32,
                            kind="ExternalOutput").ap()

    with tile.TileContext(nc) as tc, ExitStack() as ctx:
        const = ctx.enter_context(tc.tile_pool(name="const", bufs=1))
        esp = ctx.enter_context(tc.tile_pool(name="es", bufs=14))
        yp = ctx.enter_context(tc.tile_pool(name="y", bufs=3))
        smal = ctx.enter_context(tc.tile_pool(name="small", bufs=1))
        gp = ctx.enter_context(tc.tile_pool(name="g", bufs=6, space="PSUM"))
        csp = ctx.enter_context(tc.tile_pool(name="cs", bufs=2, space="PSUM"))

        # ---- staging (all DMA issue on gpsimd: 25ns vs 565ns on sync) ------
        # warm the Exp ACT table while DMAs stream
        dummy = smal.tile([1, 1], f32)
        nc.vector.memset(dummy[:], 0.0)
        nc.scalar.activation(dummy[:], dummy[:], Act.Exp)

        # [128, 2, 16]: the k-pair step must be even and 16B-aligned for
        # DoubleRow ldweights (s3_lw_dual_fp8_restrictions)
        ones_f8 = smal.tile([128, 2, 16], f8)
        nc.vector.memset(ones_f8[:], 1.0)
        ones_bf = smal.tile([128, 1], bf16)
        nc.vector.memset(ones_bf[:], 1.0)
        scratch = smal.tile([128, 64], bf16)
        nc.vector.memset(scratch[:], 1.0)

        # DMA issue costs ~0.6us per descriptor; split across two engines
        # in consumption order so group 0's slots land first.
        fc_sb = const.tile([128, JC, KC, JW], mmdt)
        maskB = const.tile([128, 4, JW], bf16)
        for s in range(8):
            nc.gpsimd.dma_start(fc_sb[:, s], fch[s])
            if s == 2:
                nc.gpsimd.dma_start(maskB[:], maskd)
        for s in range(8, JC):
            nc.sync.dma_start(fc_sb[:, s], fch[s])

        # PE p-state warmup: tiny back-to-back matmuls while DMAs land
        # (reuses a colsum PSUM buffer; in-order PE writes, no extra bank)
        wps = csp.tile([1, JW], f32, tag="cs")
        for _ in range(30):
            nc.tensor.matmul(wps[:, 0:64], ones_bf[:, 0:1], scratch[:],
                             start=True, stop=True)

        rs_all = const.tile([128, nsteps, 4], f32)
        cs_sb = const.tile([1, nsteps * JW], f32)
        # diag steps never get a colsum; zero their cs ranges once
        for td in (0, 9):
            nc.vector.memset(cs_sb[:, td * JW:(td + 1) * JW], 0.0)

        # ---- main loop -----------------------------------------------------
        pend = None  # (M, es_tiles) of previous group, colsums deferred

        def emit_colsums(M, es_tiles):
            for i, t in enumerate(M):
                if steps[t][3]:
                    continue
                cp = csp.tile([1, JW], f32, tag="cs")
                if fp8:
                    for mtp in range(2):
                        nc.tensor.matmul(cp[:], ones_f8[:, :, 0:1],
                                         es_tiles[(i, mtp)][:],
                                         start=(mtp == 0), stop=(mtp == 1),
                                         perf_mode=perf)
                else:
                    for mtp in range(2):
                        for j in range(2):
                            nc.tensor.matmul(
                                cp[:], ones_bf[:, 0:1],
                                es_tiles[(i, mtp)][:, j, :],
                                start=(mtp == 0 and j == 0),
                                stop=(mtp == 1 and j == 1))
                nc.vector.tensor_copy(cs_sb[:, t * JW:(t + 1) * JW], cp[:])
            t0, t1 = M[0], M[-1] + 1
            nc.sync.dma_start(cs_out[:, t0 * JW:t1 * JW],
                              cs_sb[:, t0 * JW:t1 * JW])

        for S, M in cfg.groups:
            es_tiles = {}
            # non-diag epilogues first so a late mask DMA can't stall ACT
            order = [i for i, t in enumerate(M) if not steps[t][3]] + \
                    [i for i, t in enumerate(M) if steps[t][3]]
            for mt in range(4):
                Gs = [gp.tile([128, JW], f32, tag="g", name=f"g{i}")
                      for i in range(len(M))]
                for k2 in range(K2):
                    ksl = slice(2 * k2, 2 * k2 + 2) if fp8 else \
                        slice(k2, k2 + 1)
                    for i, t in enumerate(M):
                        sl = steps[t][2]
                        nc.tensor.matmul(
                            Gs[i][:],
                            fc_sb[:, S, ksl, mt * 128:(mt + 1) * 128],
                            fc_sb[:, sl, ksl, :],
                            start=(k2 == 0), stop=(k2 == K2 - 1),
                            perf_mode=perf)
                for i in order:
                    t = M[i]
                    if mt % 2 == 0:
                        es_tiles[(i, mt // 2)] = esp.tile(
                            [128, 2, JW], mmdt, tag="es", name="es")
                    es = es_tiles[(i, mt // 2)]
                    if steps[t][3]:
                        y = yp.tile([128, JW], f32, tag="y")
                        nc.vector.scalar_tensor_tensor(
                            out=y[:], in0=maskB[:, mt, :],
                            scalar=-float(cfg.MASKSUB), in1=Gs[i][:],
                            op0=Alu.mult, op1=Alu.add)
                        src = y[:]
                    else:
                        src = Gs[i][:]
                    nc.scalar.activation(es[:, mt % 2, :], src, Act.Exp,
                                         scale=float(sc),
                                         accum_out=rs_all[:, t, mt:mt + 1])
            if pend is not None:
                emit_colsums(*pend)
            # ship this group's row-sum partials
            t0, t1 = M[0], M[-1] + 1
            nc.gpsimd.dma_start(rs_out[:, t0:t1, :], rs_all[:, t0:t1, :])
            pend = (M, es_tiles)
        emit_colsums(*pend)

    nc.finalize()
    return nc


# ------------------------------------------------------------ host side -----
def make_in_maps(cfg: CFG, feats_q: np.ndarray):
    JC, JW, KC = cfg.JC, cfg.JW, cfg.KC
    # X[j, p, k, n] = feats_q[j*JW + n, k*128 + p]
    X = feats_q.reshape(JC, JW, KC, 128).transpose(0, 3, 2, 1)
    idx = np.arange(128)
    maskB = np.zeros((128, 4, JW), ml_dtypes.bfloat16)
    for mt in range(4):
        maskB[idx, mt, mt * 128 + idx] = 1.0
    in_maps = []
    for c in range(cfg.NC):
        roll = [(c + s) % JC for s in range(JC)]
        in_maps.append({
            "fchunks": np.ascontiguousarray(X[roll]),
            "maskd": maskB,
        })
    return in_maps


LAST_RESULTS = None


def kernel(anchor_features, positive_features, labels):
    global LAST_RESULTS
    from concourse.bass_utils import run_bass_kernel_spmd

    cfg = CFG()
    key = (cfg.B, cfg.D, cfg.NC, cfg.MODE)
    if key not in _BUILD_CACHE:
        _BUILD_CACHE[key] = build_nc(cfg)
    nc = _BUILD_CACHE[key]

    a = np.asarray(anchor_features, dtype=np.float32)
    p = np.asarray(positive_features, dtype=np.float32)
    lab = np.asarray(labels).astype(np.float64)
    an = a / np.linalg.norm(a, axis=1, keepdims=True)
    pn = p / np.linalg.norm(p, axis=1, keepdims=True)
    cross = np.einsum("ij,ij->i", an, pn, dtype=np.float64) / cfg.TEMP
    feats = np.concatenate([an, pn], axis=0) * cfg.SCALE
    npdt = ml_dtypes.float8_e4m3 if cfg.MODE == "f8dr" else ml_dtypes.bfloat16
    feats_q = feats.astype(npdt)

    in_maps = make_in_maps(cfg, feats_q)
    trace = bool(int(os.environ.get("KERNEL_TRACE", "0")))
    res = run_bass_kernel_spmd(nc, in_maps, list(range(cfg.NC)), trace=trace)
    LAST_RESULTS = res

    # un-roll per-core partials into the global row-sum vector
    rowsum = np.zeros(cfg.R, np.float64)
    steps = cfg.steps
    for c in range(cfg.NC):
        rs = np.asarray(res.results[c]["rs_out"], np.float64)  # [128, 17, 4]
        cs = np.asarray(res.results[c]["cs_out"],
                        np.float64).reshape(len(steps), cfg.JW)
        for t, (h, g, _slot, isdiag) in enumerate(steps):
            I = (c + 8 * h) % cfg.JC
            rowsum[I * cfg.JW:(I + 1) * cfg.JW] += rs[:, t, :].T.reshape(-1)
            if not isdiag:
                J = (I + g) % cfg.JC
                rowsum[J * cfg.JW:(J + 1) * cfg.JW] += cs[t]

    lse = np.log(rowsum)
    pos2 = np.concatenate([cross, cross])
    lab2 = np.concatenate([lab, lab])
    num_pos = lab2.sum()
    loss = (lab2 * (lse - pos2)).sum() / num_pos if num_pos > 0 else 0.0
    return np.float32(loss)


# revision 15
# speedup vs baseline: 1.0287x; 1.0287x over previous
"""Contrastive loss (SimCLR-style, masked-diagonal logsumexp) on 8 Trainium2
NeuronCores via Bass/Tile.

Math (matches the jax reference):
    a = anchor / ||anchor||_row ; p = positive / ||positive||_row
    F = concat([a, p])                         # [R=2B, D]
    sim = (F F^T) / T with diagonal masked
    lse_i = log(sum_{j!=i} exp(sim_ij))
    pos_i = <a_i, p_i> / T  (duplicated for both halves)
    loss = sum_i (lse_i - pos_i) * lab_i / max(sum_i lab_i, 1)

exp(sim) is symmetric, so only the upper triangle of the 16x16 grid of
512x512 blocks is computed (136 of 256 blocks): each block (I, J) yields
row-sum partials for chunk I (ACT Exp accum) and, when I != J, column-sum
partials for chunk J (ones-vector PE matmul over the exp tile).

Distribution (uniform SPMD stream, zero collectives): core c owns chunk
rows c and c+8. Step (h, g) computes block (I, (I+g) mod 16) with
I = c + 8h, g = 0..8 for h=0 and g = 0..7 for h=1 — a circular-gap
schedule that covers every unordered block pair exactly once and gives
every core the identical 17-step instruction shape. All per-core
variation lives in the host-side chunk roll (slot s holds global chunk
(c+s) mod 16), so the stationary operands sit at fixed slots 0 and 8.

Features are L2-normalized on the host, scaled by 16 and quantized to
fp8e4 (e4m3); matmuls run in DoubleRow perf mode (2 k-subtiles per
instruction, 2x PE throughput). sim = G * (1/T)/256 rides the ACT Exp
scale. The diagonal of the two diag blocks is pushed to ~-14 in sim
units by a DVE mask subtract (exp -> ~6e-7, negligible like the
reference's exp(-1e9) = 0).

Off-diag steps are emitted in pairs sharing one 2-bank PSUM tile so a
single ACT Exp covers [128, 1024]; its accumulator then holds the SUM of
the pair's row-sums, which is all the host needs (it only ever sums
row-sum partials per phase). Exp tiles are written back as fp8 so each
column-sum is 2 DoubleRow matmuls against a ones vector (row sums come
from the ACT accumulator, which sums in f32 pre-cast).

The device ships raw per-group row/column-sum partials (~60 KB/core)
eagerly; the host un-rolls them, adds across cores, and finishes the
scalar: loss = sum(lab*(ln(rowsum) - pos))/num_pos.
"""

import os
import numpy as np
import ml_dtypes


# ---------------------------------------------------------------- config ----
class CFG:
    B = 4096
    D = 1024
    NC = 8           # cores
    JW = 512         # chunk width (one PSUM bank of f32)
    JC = 16          # number of row/col chunks (R / JW)
    KC = 8           # k-subtiles of 128
    TEMP = 0.07
    SCALE = 16.0     # fp8 pre-scale; G = SCALE^2 * cos
    MASKSUB = 512.0  # subtracted from G on the diagonal (~2x diag value)
    MODE = "f8dr"    # 'f8dr' (fp8 DoubleRow) | 'bf16'

    @property
    def R(self):
        return 2 * self.B

    # step table: t -> (h, g, slot, isdiag); slots are per-core rolled.
    @property
    def steps(self):
        out = []
        for t in range(9):
            out.append((0, t, t, t == 0))
        for t in range(8):
            out.append((1, t, 8 + t, t == 0))
        return out

    # emission groups: (stationary slot, step list); diag steps ride alone,
    # off-diag steps in pairs sharing one 2-bank PSUM tile / ACT instruction
    @property
    def groups(self):
        return [
            (0, [0]), (0, [1, 2]), (0, [3, 4]), (0, [5, 6]), (0, [7, 8]),
            (8, [9]), (8, [10, 11]), (8, [12, 13]), (8, [14, 15]), (8, [16]),
        ]


_BUILD_CACHE = {}


# ----------------------------------------------------------------- build ----
def build_nc(cfg: CFG):
    import concourse.bass as bass  # noqa: F401  (AP helpers live here)
    import concourse.tile as tile
    from contextlib import ExitStack
    from concourse import bacc, mybir

    f32 = mybir.dt.float32
    bf16 = mybir.dt.bfloat16
    f8 = mybir.dt.float8e4
    Act = mybir.ActivationFunctionType
    Alu = mybir.AluOpType

    JW, JC, KC = cfg.JW, cfg.JC, cfg.KC
    fp8 = cfg.MODE == "f8dr"
    mmdt = f8 if fp8 else bf16
    K2 = KC // 2 if fp8 else KC           # matmul k-steps per block
    perf = mybir.MatmulPerfMode.DoubleRow if fp8 else None
    sc = (1.0 / cfg.TEMP) / (cfg.SCALE * cfg.SCALE)
    steps = cfg.steps
    groups = cfg.groups
    nsteps = len(steps)
    ngroups = len(groups)

    nc = bacc.Bacc("TRN2", target_bir_lowering=False, debug=False,
                   num_devices=cfg.NC)

    fch = nc.dram_tensor("fchunks", [JC, 128, KC, JW], mmdt,
                         kind="ExternalInput").ap()
    maskd = nc.dram_tensor("maskd", [128, 4, JW], bf16,
                           kind="ExternalInput").ap()
    rs_out = nc.dram_tensor("rs_out", [128, ngroups, 4], f32,
                            kind="ExternalOutput").ap()
    cs_out = nc.dram_tensor("cs_out", [1, nsteps * JW], f32,
                            kind="ExternalOutput").ap()

    with tile.TileContext(nc) as tc, ExitStack() as ctx:
        const = ctx.enter_context(tc.tile_pool(name="const", bufs=1))
        esp2 = ctx.enter_context(tc.tile_pool(name="es2", bufs=4))
        esp1 = ctx.enter_context(tc.tile_pool(name="es1", bufs=2))
        yp = ctx.enter_context(tc.tile_pool(name="y", bufs=3))
        smal = ctx.enter_context(tc.tile_pool(name="small", bufs=1))
        gp = ctx.enter_context(tc.tile_pool(name="g", bufs=3, space="PSUM"))
        csp = ctx.enter_context(tc.tile_pool(name="cs", bufs=2, space="PSUM"))

        # ---- staging -------------------------------------------------------
        # warm the Exp ACT table while DMAs stream
        dummy = smal.tile([1, 1], f32)
        nc.vector.memset(dummy[:], 0.0)
        nc.scalar.activation(dummy[:], dummy[:], Act.Exp)

        # [128, 2, 16]: the k-pair step must be even and 16B-aligned for
        # DoubleRow ldweights (s3_lw_dual_fp8_restrictions)
        ones_f8 = smal.tile([128, 2, 16], f8)
        nc.vector.memset(ones_f8[:], 1.0)
        ones_bf = smal.tile([128, 1], bf16)
        nc.vector.memset(ones_bf[:], 1.0)
        scratch = smal.tile([128, 64], bf16)
        nc.vector.memset(scratch[:], 1.0)

        # DMA issue costs ~0.6us per descriptor; alternate two issue engines
        # in consumption order (slot 0 halves first, mask early for the
        # group-0 diag, then slots in group order).
        fc_sb = const.tile([128, JC, KC, JW], mmdt)
        maskB = const.tile([128, 4, JW], bf16)
        nc.gpsimd.dma_start(fc_sb[:, 0, 0:4, :], fch[0, :, 0:4, :])
        nc.sync.dma_start(fc_sb[:, 0, 4:8, :], fch[0, :, 4:8, :])
        nc.gpsimd.dma_start(maskB[:], maskd)
        for s in range(1, JC):
            eng = nc.sync if s % 2 else nc.gpsimd
            eng.dma_start(fc_sb[:, s], fch[s])

        # PE p-state warmup: tiny back-to-back matmuls while DMAs land
        # (reuses a colsum PSUM buffer; in-order PE writes, no extra bank)
        wps = csp.tile([1, JW], f32, tag="cs")
        for _ in range(12):
            nc.tensor.matmul(wps[:, 0:64], ones_bf[:, 0:1], scratch[:],
                             start=True, stop=True)

        rs_all = const.tile([128, ngroups, 4], f32)
        cs_sb = const.tile([1, nsteps * JW], f32)
        # diag steps never get a colsum; zero their cs ranges once
        for td in (0, 9):
            nc.vector.memset(cs_sb[:, td * JW:(td + 1) * JW], 0.0)

        # ---- main loop -----------------------------------------------------
        pend = None  # (M, es_g) of previous group, colsums deferred

        def emit_colsums(M, es_g):
            for i, t in enumerate(M):
                if steps[t][3]:
                    continue
                cp = csp.tile([1, JW], f32, tag="cs")
                if fp8:
                    for mtp in (0, 2):
                        nc.tensor.matmul(cp[:], ones_f8[:, :, 0:1],
                                         es_g[:, mtp:mtp + 2, i, :],
                                         start=(mtp == 0), stop=(mtp == 2),
                                         perf_mode=perf)
                else:
                    for mt in range(4):
                        nc.tensor.matmul(
                            cp[:], ones_bf[:, 0:1], es_g[:, mt, i, :],
                            start=(mt == 0), stop=(mt == 3))
                nc.vector.tensor_copy(cs_sb[:, t * JW:(t + 1) * JW], cp[:])
            t0, t1 = M[0], M[-1] + 1
            nc.sync.dma_start(cs_out[:, t0 * JW:t1 * JW],
                              cs_sb[:, t0 * JW:t1 * JW])

        for gidx, (S, M) in enumerate(groups):
            nM = len(M)
            esp = esp2 if nM == 2 else esp1
            es_g = esp.tile([128, 4, nM, JW], mmdt, tag="es", name="es")
            for mt in range(4):
                Gt = gp.tile([128, 2, JW], f32, tag="g")
                for k2 in range(K2):
                    ksl = slice(2 * k2, 2 * k2 + 2) if fp8 else \
                        slice(k2, k2 + 1)
                    for i, t in enumerate(M):
                        sl = steps[t][2]
                        nc.tensor.matmul(
                            Gt[:, i, :],
                            fc_sb[:, S, ksl, mt * 128:(mt + 1) * 128],
                            fc_sb[:, sl, ksl, :],
                            start=(k2 == 0), stop=(k2 == K2 - 1),
                            perf_mode=perf)
                if nM == 2:
                    # one Exp over both banks; accum = sum of the pair's
                    # row-sums, which is all the host needs
                    nc.scalar.activation(es_g[:, mt, :, :], Gt[:],
                                         Act.Exp, scale=float(sc),
                                         accum_out=rs_all[:, gidx, mt:mt + 1])
                else:
                    t = M[0]
                    if steps[t][3]:
                        y = yp.tile([128, JW], f32, tag="y")
                        nc.vector.scalar_tensor_tensor(
                            out=y[:], in0=maskB[:, mt, :],
                            scalar=-float(cfg.MASKSUB), in1=Gt[:, 0, :],
                            op0=Alu.mult, op1=Alu.add)
                        src = y[:]
                    else:
                        src = Gt[:, 0, :]
                    nc.scalar.activation(es_g[:, mt, 0, :], src, Act.Exp,
                                         scale=float(sc),
                                         accum_out=rs_all[:, gidx, mt:mt + 1])
            if pend is not None:
                emit_colsums(*pend)
            nc.gpsimd.dma_start(rs_out[:, gidx:gidx + 1, :],
                                rs_all[:, gidx:gidx + 1, :])
            pend = (M, es_g)
        emit_colsums(*pend)

    nc.finalize()
    return nc


# ------------------------------------------------------------ host side -----
def make_in_maps(cfg: CFG, feats_q: np.ndarray):
    JC, JW, KC = cfg.JC, cfg.JW, cfg.KC
    # X[j, p, k, n] = feats_q[j*JW + n, k*128 + p]
    X = feats_q.reshape(JC, JW, KC, 128).transpose(0, 3, 2, 1)
    idx = np.arange(128)
    maskB = np.zeros((128, 4, JW), ml_dtypes.bfloat16)
    for mt in range(4):
        maskB[idx, mt, mt * 128 + idx] = 1.0
    in_maps = []
    for c in range(cfg.NC):
        roll = [(c + s) % JC for s in range(JC)]
        in_maps.append({
            "fchunks": np.ascontiguousarray(X[roll]),
            "maskd": maskB,
        })
    return in_maps


LAST_RESULTS = None


def kernel(anchor_features, positive_features, labels):
    global LAST_RESULTS
    from concourse.bass_utils import run_bass_kernel_spmd

    cfg = CFG()
    key = (cfg.B, cfg.D, cfg.NC, cfg.MODE)
    if key not in _BUILD_CACHE:
        _BUILD_CACHE[key] = build_nc(cfg)
    nc = _BUILD_CACHE[key]

    a = np.asarray(anchor_features, dtype=np.float32)
    p = np.asarray(positive_features, dtype=np.float32)
    lab = np.asarray(labels).astype(np.float64)
    an = a / np.linalg.norm(a, axis=1, keepdims=True)
    pn = p / np.linalg.norm(p, axis=1, keepdims=True)
    cross = np.einsum("ij,ij->i", an, pn, dtype=np.float64) / cfg.TEMP
    feats = np.concatenate([an, pn], axis=0) * cfg.SCALE
    npdt = ml_dtypes.float8_e4m3 if cfg.MODE == "f8dr" else ml_dtypes.bfloat16
    feats_q = feats.astype(npdt)

    in_maps = make_in_maps(cfg, feats_q)
    trace = bool(int(os.environ.get("KERNEL_TRACE", "0")))
    res = run_bass_kernel_spmd(nc, in_maps, list(range(cfg.NC)), trace=trace)
    LAST_RESULTS = res

    # un-roll per-core partials into the global row-sum vector
    rowsum = np.zeros(cfg.R, np.float64)
    steps = cfg.steps
    for c in range(cfg.NC):
        rs = np.asarray(res.results[c]["rs_out"], np.float64)  # [128, 10, 4]
        cs = np.asarray(res.results[c]["cs_out"],
                        np.float64).reshape(len(steps), cfg.JW)
        for gidx, (S, M) in enumerate(cfg.groups):
            I = (c + (8 if S else 0)) % cfg.JC
            # group accum = sum of its steps' row-sum partials for chunk I
            rowsum[I * cfg.JW:(I + 1) * cfg.JW] += rs[:, gidx, :].T.reshape(-1)
            for t in M:
                h, g, _slot, isdiag = steps[t]
                if not isdiag:
                    J = (I + g) % cfg.JC
                    rowsum[J * cfg.JW:(J + 1) * cfg.JW] += cs[t]

    lse = np.log(rowsum)
    pos2 = np.concatenate([cross, cross])
    lab2 = np.concatenate([lab, lab])
    num_pos = lab2.sum()
    loss = (lab2 * (lse - pos2)).sum() / num_pos if num_pos > 0 else 0.0
    return np.float32(loss)
